# revision 1
# baseline (speedup 1.0000x reference)
"""Trainium2 Bass kernel for nn_DifferentialMaxtree.

Strategy (8 NeuronCores, data-parallel over the 32 (b,n) trees, 4 per core):
  1. Features/logits/sigmoid/w = diff*score on ACT+DVE (fp32, elementwise).
  2. Path sums via pointer doubling. The gather engine is GPSIMD ap_gather
     over quarter-tables replicated per Q7 core (fp32, exact); the pointer
     chains (pure int index data) are precomputed host-side per doubling
     iteration, so the device only gathers s-values and accumulates with
     0/1 masks (bitwise-exact vs. the reference recurrence).
  3. Pixel lookup with the same gather+select machinery from the val table.

Layout: component e of a tree lives at wrap position (p, f):
  e = 8192*(p//16) + 16*f + (p%16), so that request i of Q7 core k reads its
  index at partition (16k + i%16), offset i//16 == the same (p, f) spot.
"""

import numpy as np

import concourse.bass as bass
import concourse.bacc as bacc
import concourse.mybir as mybir
import concourse.tile as tile
from concourse.bass_utils import run_bass_kernel_spmd

f32 = mybir.dt.float32
i16 = mybir.dt.int16
i32 = mybir.dt.int32
Alu = mybir.AluOpType
Act = mybir.ActivationFunctionType

CFG = dict(
    B=4, N=8, H=512, W=512, C=65536,
    NCORES=8,   # NeuronCores
    TPC=4,      # trees per NeuronCore
    P=128,
    EPS=1e-10,
    SCALING=10.0,
)


def _dims(cfg):
    C, P = cfg["C"], cfg["P"]
    J = C // P              # wrap free dim (components per partition)
    NE = C // 8             # eighth-table entries per channel
    NPX = cfg["H"] * cfg["W"]          # pixels per tree
    PIXCH = min(C, NPX)                # pixels per gather instruction
    return J, NE, NPX, PIXCH


def _wrap16(arr_percore):
    """[8, M] per-core request streams -> [128, M//16] idx tiles.

    Request i of Q7 core k reads its index at partition 16k + i%16,
    free offset i//16."""
    out = np.empty((128, arr_percore.shape[1] // 16), arr_percore.dtype)
    for k in range(8):
        out[16 * k:16 * k + 16] = arr_percore[k].reshape(-1, 16).T
    return out


# ---------------------------------------------------------------- host prep


def _host_prep(cfg, diff, attrs, weight, bias, parent, pix2cc):
    B, N, C, P = cfg["B"], cfg["N"], cfg["C"], cfg["P"]
    NCORES, TPC = cfg["NCORES"], cfg["TPC"]
    J, NE, NPX, PIXCH = _dims(cfg)

    # pointer chains (host: pure index bookkeeping; all float math on device)
    pz = np.concatenate([parent, np.full((B, N, 1), C, np.int32)], axis=-1)
    chains = []
    cur = pz.copy()
    k_iters = 17
    for k in range(17):
        chains.append(cur[..., :C].copy())      # gather map of iteration k
        if (cur == C).all():
            k_iters = k
            break
        cur = np.take_along_axis(cur, cur, axis=-1)
    else:
        chains.append(None)
    k_iters = min(k_iters, 17)
    if k_iters == 0:
        k_iters = 1  # degenerate: all parents already sentinel; run 1 no-op iter
    chains = chains[:k_iters]

    npix_instr = TPC * (NPX // PIXCH)
    in_maps = []
    for core in range(NCORES):
        attrs_w = np.empty((TPC, P, J, 15), np.float32)
        diff_w = np.empty((TPC, P, J), np.float32)
        wgtB = np.empty((P, TPC, 17), np.float32)
        biasB = np.empty((P, TPC), np.float32)
        c_idx = np.empty((TPC, k_iters, P, J), np.int16)
        c_qsel = np.empty((TPC, k_iters, P, J), np.float32)
        p_idx = np.empty((npix_instr, P, PIXCH // P), np.int16)
        p_qsel = np.empty((npix_instr, P, PIXCH // P), np.float32)
        for j in range(TPC):
            t = core * TPC + j
            b, n = t // N, t % N
            attrs_w[j] = attrs[b, n].reshape(P, J, 15)
            diff_w[j] = diff[b, n].reshape(P, J)
            wgtB[:, j, :] = weight[n, :, 0][None, :]
            biasB[:, j] = bias[n, 0]
            for k in range(k_iters):
                g = chains[k][b, n]              # int32 in [0, C], by component
                c_idx[j, k] = _wrap16((g % NE).astype(np.int16).reshape(8, -1))
                c_qsel[j, k] = (g // NE).astype(np.float32).reshape(P, J)
            pix = pix2cc[b, n].reshape(-1)
            nch = NPX // PIXCH
            for c in range(nch):
                blk = pix[c * PIXCH:(c + 1) * PIXCH]
                p_idx[j * nch + c] = _wrap16(
                    (blk % NE).astype(np.int16).reshape(8, -1))
                p_qsel[j * nch + c] = (blk // NE).astype(np.float32).reshape(
                    P, PIXCH // P)
        in_maps.append(dict(
            attrs_w=attrs_w, diff_w=diff_w, wgtB=wgtB, biasB=biasB,
            c_idx=c_idx, c_qsel=c_qsel, p_idx=p_idx, p_qsel=p_qsel,
        ))
    return in_maps, k_iters, npix_instr


def _host_assemble(cfg, results):
    B, N, C, P = cfg["B"], cfg["N"], cfg["C"], cfg["P"]
    NCORES, TPC = cfg["NCORES"], cfg["TPC"]
    H, W = cfg["H"], cfg["W"]
    J, NE, NPX, PIXCH = _dims(cfg)
    out = np.empty((B, N, H, W), np.float32)
    nch = NPX // PIXCH
    for core in range(NCORES):
        po = results[core]["pixout"]  # [npix_instr, P, PIXCH//P]
        for j in range(TPC):
            t = core * TPC + j
            b, n = t // N, t % N
            out[b, n] = po[j * nch:(j + 1) * nch].reshape(H, W)
    return out


# ------------------------------------------------------------- device build


def _build(cfg, k_iters, npix_instr, use_pstride):
    P, TPC, EPS = cfg["P"], cfg["TPC"], cfg["EPS"]
    C = cfg["C"]
    J, NE, NPX, PIXCH = _dims(cfg)
    JP = PIXCH // P          # pixel idx free dim
    NI = 8 * JP * 16 // 16   # num_idxs per core for pixel instr == PIXCH//8
    NIC = PIXCH // 8         # per-core num_idxs (pixel)
    NCC = C // 8             # per-core num_idxs (chase) == 16*J... C//8
    NQ = 8

    nc = bacc.Bacc("TRN2", target_bir_lowering=False, num_devices=cfg["NCORES"])
    attrs_w = nc.dram_tensor("attrs_w", [TPC, P, J, 15], f32, kind="ExternalInput")
    diff_w = nc.dram_tensor("diff_w", [TPC, P, J], f32, kind="ExternalInput")
    wgtB = nc.dram_tensor("wgtB", [P, TPC, 17], f32, kind="ExternalInput")
    biasB = nc.dram_tensor("biasB", [P, TPC], f32, kind="ExternalInput")
    c_idx = nc.dram_tensor("c_idx", [TPC, k_iters, P, J], i16, kind="ExternalInput")
    c_qsel = nc.dram_tensor("c_qsel", [TPC, k_iters, P, J], f32, kind="ExternalInput")
    p_idx = nc.dram_tensor("p_idx", [npix_instr, P, JP], i16, kind="ExternalInput")
    p_qsel = nc.dram_tensor("p_qsel", [npix_instr, P, JP], f32, kind="ExternalInput")
    pixout = nc.dram_tensor("pixout", [npix_instr, P, JP], f32, kind="ExternalOutput")

    with tile.TileContext(nc) as tc:
        with (
            tc.tile_pool(name="sb", bufs=1) as pool0,
            tc.tile_pool(name="dr", bufs=1, space="DRAM") as dpool,
        ):
            wg = pool0.tile([P, TPC, 17], f32, tag="wg")
            nc.sync.dma_start(wg[:], wgtB[:, :, :])
            bi = pool0.tile([P, TPC], f32, tag="bi")
            nc.sync.dma_start(bi[:], biasB[:, :])
            epsb = pool0.tile([P, 1], f32, tag="epsb")
            nc.vector.memset(epsb[:], 1e-10)
            hpib = pool0.tile([P, 1], f32, tag="hpib")
            nc.vector.memset(hpib[:], float(np.pi / 2))
            qconst = []
            for q in range(NQ):
                cq = pool0.tile([P, 1], f32, tag=f"qc{q}")
                nc.vector.memset(cq[:], float(q))
                qconst.append(cq)

            # ---------------- features -> s_j (= w of tree j), fp32
            s_tiles = []
            with tc.tile_pool(name="sbf", bufs=1) as poolf:
                for j in range(TPC):
                    at = poolf.tile([P, J, 15], f32, tag="at")
                    nc.sync.dma_start(at[:], attrs_w[j])
                    df = poolf.tile([P, J], f32, tag="df")
                    nc.sync.dma_start(df[:], diff_w[j])
                    t9 = poolf.tile([P, J, 9], f32, tag="t9")
                    s9 = poolf.tile([P, J, 9], f32, tag="s9")
                    t1 = poolf.tile([P, J], f32, tag="t1")
                    t2 = poolf.tile([P, J], f32, tag="t2")
                    tm = poolf.tile([P, J], f32, tag="tm")
                    lg = poolf.tile([P, J], f32, tag="lg")

                    def wgb(kf, jj=j):
                        return wg[:, jj, kf:kf + 1].to_broadcast([P, J])

                    def acc(feat_ap, kf, first=False):
                        if first:
                            nc.vector.tensor_tensor(
                                out=lg[:], in0=feat_ap, in1=wgb(kf), op=Alu.mult)
                        else:
                            nc.vector.tensor_tensor(
                                out=tm[:], in0=feat_ap, in1=wgb(kf), op=Alu.mult)
                            nc.vector.tensor_tensor(
                                out=lg[:], in0=lg[:], in1=tm[:], op=Alu.add)

                    for k in range(4):
                        acc(at[:, :, k], k, first=(k == 0))
                    # area = log(a4)
                    nc.scalar.activation(t1[:], at[:, :, 4], Act.Ln)
                    acc(t1[:], 4)
                    # tail: log(|a|+eps)*sign(a), attrs 6..14
                    nc.scalar.activation(t9[:], at[:, :, 6:15], Act.Abs)
                    nc.scalar.activation(t9[:], t9[:], Act.Ln, bias=epsb[:, :])
                    nc.scalar.activation(s9[:], at[:, :, 6:15], Act.Sign)
                    nc.vector.tensor_tensor(
                        out=t9[:], in0=t9[:], in1=s9[:], op=Alu.mult)
                    for k in range(9):
                        acc(t9[:, :, k], 5 + k)
                    # lshape = sqrt(a7) / (sqrt(a6) + eps)
                    nc.scalar.activation(t1[:], at[:, :, 6], Act.Sqrt)
                    nc.scalar.activation(t1[:], t1[:], Act.Copy, bias=EPS)
                    nc.vector.reciprocal(t1[:], t1[:])
                    nc.scalar.activation(t2[:], at[:, :, 7], Act.Sqrt)
                    nc.vector.tensor_tensor(
                        out=t2[:], in0=t2[:], in1=t1[:], op=Alu.mult)
                    acc(t2[:], 14)
                    # cos, sin
                    nc.scalar.activation(t1[:], at[:, :, 5], Act.Sin,
                                         bias=hpib[:, :])
                    acc(t1[:], 15)
                    nc.scalar.activation(t2[:], at[:, :, 5], Act.Sin)
                    acc(t2[:], 16)
                    # + bias; sigmoid; w = diff * score
                    nc.vector.tensor_tensor(
                        out=lg[:], in0=lg[:],
                        in1=bi[:, j:j + 1].to_broadcast([P, J]), op=Alu.add)
                    nc.scalar.activation(lg[:], lg[:], Act.Sigmoid)
                    s_j = pool0.tile([P, J], f32, tag=f"s{j}")
                    nc.vector.tensor_tensor(
                        out=s_j[:], in0=lg[:], in1=df[:], op=Alu.mult)
                    s_tiles.append(s_j)

            # helpers -------------------------------------------------------
            # State images are stored 8x replicated in DRAM: copy r holds the
            # flat component-major image, so the concatenation viewed as
            # [128, NE] is exactly the per-Q7-core table layout (partition
            # 16r+c holds content c) and a table load is ONE contiguous DMA.
            def write_eorder(dram_ap, sb_ap, nelem):
                dv = dram_ap.rearrange("(r pf) -> r pf", r=16)
                for r in range(16):
                    nc.scalar.dma_start(
                        dv[r].rearrange("(p f) -> p f", p=P), sb_ap[:, :])

            def load_table(tbl, dram_flat, nquart, active_rows):
                nc.sync.dma_start(
                    tbl[:], dram_flat.rearrange("(p e) -> p e", p=P))

            def gather_select(tbl, idx_ap, qsel_ap, out_tile, jf, num_idx_core,
                              tagsuffix=""):
                # gather + co-locate roundtrip + NQ-way masked select.
                # Request i of a core handles state (p_local=i//jf, f=i%jf), so
                # gathered column-runs per state partition are contiguous.
                g = pool.tile([P, num_idx_core], f32, tag="g" + tagsuffix, bufs=2)
                nc.gpsimd.ap_gather(
                    out_ap=g[:], in_ap=tbl[:], idxs_ap=idx_ap,
                    channels=P, num_elems=NE, d=1, num_idxs=num_idx_core)
                dc = dpool.tile([8, NQ * num_idx_core], f32, tag="dc" + tagsuffix,
                               bufs=2)
                for k2 in range(8):
                    nc.scalar.dma_start(
                        dc[k2].rearrange("(q i) -> q i", q=NQ),
                        g[16 * k2:16 * k2 + NQ, :])
                cand = pool.tile([P, NQ, jf], f32, tag="cand" + tagsuffix, bufs=2)
                for k2 in range(8):
                    nc.sync.dma_start(
                        cand[16 * k2:16 * k2 + 16, :, :],
                        dc[k2].rearrange("(q p f) -> p q f", p=16, q=NQ))
                mq = pool.tile([P, jf], f32, tag="mq" + tagsuffix)
                tt = pool.tile([P, jf], f32, tag="tt" + tagsuffix)
                for q in range(NQ):
                    nc.vector.tensor_tensor(
                        out=mq[:], in0=qsel_ap,
                        in1=qconst[q][:, :].to_broadcast([P, jf]), op=Alu.is_equal)
                    if q == 0:
                        nc.vector.tensor_tensor(
                            out=out_tile[:], in0=cand[:, q, :], in1=mq[:],
                            op=Alu.mult)
                    else:
                        nc.vector.tensor_tensor(
                            out=tt[:], in0=cand[:, q, :], in1=mq[:], op=Alu.mult)
                        nc.vector.tensor_tensor(
                            out=out_tile[:], in0=out_tile[:], in1=tt[:], op=Alu.add)

            _es = __import__("contextlib").ExitStack()
            pool = _es.enter_context(tc.tile_pool(name="sbc", bufs=1))
            # table buffers: allocated once; rows 4..15 of each 16-block are
            # never read (gather returns garbage there, masked off), but are
            # memset once so simulation sees initialized memory.
            tbl_bufs = []
            for tb in range(2):
                tbt = pool.tile([P, NE], f32, tag=f"tblb{tb}")
                nc.vector.memset(tbt[:], 0.0)
                tbl_bufs.append(tbt)

            # ---------------- chase (trees interleaved two-wide so one
            # tree's gather hides the other's rebuild chain)
            d2s = []
            for j in range(TPC):
                d2 = dpool.tile([16 * C], f32, tag=f"d2_{j}", bufs=2)
                write_eorder(d2[:], s_tiles[j][:], C)
                d2s.append(d2)
            rows = list(range(NQ))
            for k in range(k_iters):
                for j in range(TPC):
                    s_j = s_tiles[j]
                    tbl = tbl_bufs[j % 2]
                    load_table(tbl, d2s[j][:], NQ, rows)
                    ci_t = pool.tile([P, J], i16, tag="cidx", bufs=3)
                    nc.scalar.dma_start(ci_t[:], c_idx[j, k])
                    cq_t = pool.tile([P, J], f32, tag="cqs", bufs=3)
                    nc.scalar.dma_start(cq_t[:], c_qsel[j, k])
                    sel = pool.tile([P, J], f32, tag="sel", bufs=2)
                    gather_select(tbl, ci_t[:], cq_t[:], sel, J, C // 8)
                    nc.vector.tensor_tensor(
                        out=s_j[:], in0=s_j[:], in1=sel[:], op=Alu.add)
                    if k < k_iters - 1:
                        d2 = dpool.tile([16 * C], f32, tag=f"d2_{j}", bufs=2)
                        write_eorder(d2[:], s_j[:], C)
                        d2s[j] = d2
            for j in range(TPC):
                s_j = s_tiles[j]
                # val = s / SCALING
                nc.vector.tensor_scalar_mul(s_j[:], s_j[:], 1.0 / cfg["SCALING"])
                vd = dpool.tile([16 * C], f32, tag=f"val{j}")
                write_eorder(vd[:], s_j[:], C)
                s_tiles[j] = (s_j, vd)

            # ---------------- pixels
            nch = NPX // PIXCH
            for j in range(TPC):
                vd = s_tiles[j][1]
                ptbl = tbl_bufs[j % 2]
                load_table(ptbl, vd[:], NQ, rows)
                for c in range(nch):
                    i = j * nch + c
                    pi_t = pool.tile([P, JP], i16, tag="pidx", bufs=2)
                    nc.scalar.dma_start(pi_t[:], p_idx[i])
                    pq_t = pool.tile([P, JP], f32, tag="pqs", bufs=2)
                    nc.scalar.dma_start(pq_t[:], p_qsel[i])
                    po = pool.tile([P, JP], f32, tag="po", bufs=2)
                    gather_select(ptbl, pi_t[:], pq_t[:], po, JP, NIC)
                    nc.sync.dma_start(pixout[i], po[:])
            _es.close()

    nc.compile()
    return nc


_CACHE = {}
TRACE = False
LAST_RESULT = None


def _get_nc(cfg, k_iters, npix_instr, use_pstride=False):
    key = (cfg["C"], cfg["H"], k_iters, npix_instr, use_pstride)
    if key not in _CACHE:
        _CACHE[key] = _build(cfg, k_iters, npix_instr, use_pstride)
    return _CACHE[key]


def kernel(diff, attrs, weight, bias, parent, pix2cc):
    cfg = CFG
    diff = np.ascontiguousarray(np.asarray(diff, np.float32))
    attrs = np.ascontiguousarray(np.asarray(attrs, np.float32))
    weight = np.ascontiguousarray(np.asarray(weight, np.float32))
    bias = np.ascontiguousarray(np.asarray(bias, np.float32))
    parent = np.ascontiguousarray(np.asarray(parent, np.int32))
    pix2cc = np.ascontiguousarray(np.asarray(pix2cc, np.int32))

    in_maps, k_iters, npix_instr = _host_prep(
        cfg, diff, attrs, weight, bias, parent, pix2cc)
    nc = _get_nc(cfg, k_iters, npix_instr)
    res = run_bass_kernel_spmd(
        nc, in_maps, core_ids=list(range(cfg["NCORES"])), trace=TRACE)
    global LAST_RESULT
    LAST_RESULT = res
    return _host_assemble(cfg, res.results)



# revision 4
# speedup vs baseline: 1.2480x; 1.2480x over previous
"""Trainium2 Bass kernel for nn_DifferentialMaxtree (v2).

Strategy (8 NeuronCores, data-parallel over the 32 (b,n) trees, 4 per core):
  1. Features/logits/sigmoid/w = diff*score on ACT+DVE, computed directly in
     the "slot" layout (host pre-permutes attrs/diff), so w lands in state
     layout with zero movement.
  2. Path sums via pointer doubling (host precomputes the per-iteration
     pointer chains; device does all float math).  Per iteration:
       - state -> DRAM dump (256KB, contiguous) -> 16x-replicated gather
         table [128, 8192] via contiguous 32KB-run loads,
       - gpsimd ap_gather (8192 idx/core),
       - one DVE 32x32 stream-transpose colocates每 request's 8 candidates
         onto the request's own partition,
       - masked select (is_equal + mult + reduce) lands *exactly* in state
         layout -> elementwise add.  No DRAM roundtrips for colocation.
  3. Pixel lookup reuses the same machinery (1 table per tree, 4 chunks of
     65536 pixels); slot<->pixel mapping is identity so host assembly is a
     reshape.

Slot layout: slot (p, x) of a tree holds component
    e(p, x) = 8192*(2*(p//32) + x%2) + 512*(p%16) + (x & ~1) + (p//16)%2
which makes (a) the gather-idx tile natural-layout, (b) the select output
land in state layout, and (c) the table rebuild fully contiguous.
"""

import numpy as np

import concourse.bass as bass
import concourse.bacc as bacc
import concourse.mybir as mybir
import concourse.tile as tile
from concourse.bass_utils import run_bass_kernel_spmd

f32 = mybir.dt.float32
i16 = mybir.dt.int16
i32 = mybir.dt.int32
u8 = mybir.dt.uint8
Alu = mybir.AluOpType
Act = mybir.ActivationFunctionType

CFG = dict(
    B=4, N=8, H=512, W=512, C=65536,
    NCORES=8,   # NeuronCores
    TPC=4,      # trees per NeuronCore
    P=128,
    EPS=1e-10,
    SCALING=10.0,
)

P = 128
C = 65536
J = 512          # free size of state image
NE = C // 8      # table row length (one chunk)
NPIXCH = 4       # pixel chunks per tree (65536 pixels each)


def _slot_component_map():
    """e(p, x): component id held at state slot (p, x)."""
    p = np.arange(P, dtype=np.int64)[:, None]
    x = np.arange(J, dtype=np.int64)[None, :]
    e = (8192 * (2 * (p // 32) + (x % 2)) + 512 * (p % 16)
         + (x & ~1) + (p // 16) % 2)
    return e  # [128, 512]


def _rem(g):
    """Within-chunk table offset of component/sentinel g (sentinel -> 0)."""
    g = g.astype(np.int64)
    return (512 * ((g // 512) % 16) + ((g % 512) & ~1)
            + (g // 8192) % 2).astype(np.int16)


def _chunk(g):
    """Table chunk of g in [0,8); sentinel C maps to 8 (selects nothing)."""
    g = g.astype(np.int64)
    return (2 * (g // 16384) + (g % 2)).astype(np.uint8)


def _req_positions():
    """Where slot (p, x)'s gather request reads its index: (P_i, F_i)."""
    p = np.arange(P, dtype=np.int64)[:, None]
    x = np.arange(J, dtype=np.int64)[None, :]
    k = 2 * (p // 32) + (x % 2)
    P_i = 16 * k + (p % 16)
    F_i = (x & ~1) + (p // 16) % 2
    return P_i, F_i


# ---------------------------------------------------------------- host prep


def _host_prep(cfg, diff, attrs, weight, bias, parent, pix2cc):
    B, N = cfg["B"], cfg["N"]
    NCORES, TPC = cfg["NCORES"], cfg["TPC"]

    e_slot = _slot_component_map()

    # pointer chains (host: pure index bookkeeping; all float math on device)
    pz = np.concatenate([parent, np.full((B, N, 1), C, np.int32)], axis=-1)
    chains = []
    cur = pz.copy()
    k_iters = 17
    for k in range(17):
        if (cur == C).all():
            k_iters = k
            break
        chains.append(cur[..., :C].copy())
        cur = np.take_along_axis(cur, cur, axis=-1)
    k_iters = len(chains)
    if k_iters == 0:
        chains.append(pz[..., :C].copy())
        k_iters = 1

    P_i, F_i = _req_positions()

    in_maps = []
    for core in range(NCORES):
        attrs_sw = np.empty((TPC, P, J, 15), np.float32)
        diff_sw = np.empty((TPC, P, J), np.float32)
        wgtB = np.empty((P, TPC, 17), np.float32)
        biasB = np.empty((P, TPC), np.float32)
        c_idx = np.empty((TPC, k_iters, P, J), np.int16)
        c_qsel = np.empty((TPC, k_iters, P, J), np.uint8)
        p_idx = np.empty((TPC, NPIXCH, P, J), np.int16)
        p_qsel = np.empty((TPC, NPIXCH, P, J), np.uint8)
        for j in range(TPC):
            t = core * TPC + j
            b, n = t // N, t % N
            attrs_sw[j] = attrs[b, n][e_slot]
            diff_sw[j] = diff[b, n][e_slot]
            wgtB[:, j, :] = weight[n, :, 0][None, :]
            biasB[:, j] = bias[n, 0]
            for k in range(k_iters):
                g = chains[k][b, n]                      # by component, [C]
                c_idx[j, k] = _rem(g).reshape(P, J)      # natural layout
                c_qsel[j, k] = _chunk(g)[e_slot]         # slot layout
            pix = pix2cc[b, n].reshape(-1)
            for c in range(NPIXCH):
                gp = pix[c * C:(c + 1) * C].reshape(P, J)  # slot (p,x)=pixel
                p_qsel[j, c] = _chunk(gp)
                pi = np.empty((P, J), np.int16)
                pi[P_i, F_i] = _rem(gp)
                p_idx[j, c] = pi
        in_maps.append(dict(
            attrs_sw=attrs_sw, diff_sw=diff_sw, wgtB=wgtB, biasB=biasB,
            c_idx=c_idx, c_qsel=c_qsel, p_idx=p_idx, p_qsel=p_qsel,
        ))
    return in_maps, k_iters


def _host_assemble(cfg, results):
    B, N = cfg["B"], cfg["N"]
    NCORES, TPC = cfg["NCORES"], cfg["TPC"]
    H, W = cfg["H"], cfg["W"]
    out = np.empty((B, N, H, W), np.float32)
    for core in range(NCORES):
        po = results[core]["pixout"]  # [TPC, NPIXCH, P, J]
        for j in range(TPC):
            t = core * TPC + j
            b, n = t // N, t % N
            out[b, n] = po[j].reshape(H, W)
    return out


# ------------------------------------------------------------- device build


def _build(cfg, k_iters):
    TPC, EPS = cfg["TPC"], cfg["EPS"]

    nc = bacc.Bacc("TRN2", target_bir_lowering=False, num_devices=cfg["NCORES"])
    attrs_sw = nc.dram_tensor("attrs_sw", [TPC, P, J, 15], f32, kind="ExternalInput")
    diff_sw = nc.dram_tensor("diff_sw", [TPC, P, J], f32, kind="ExternalInput")
    wgtB = nc.dram_tensor("wgtB", [P, TPC, 17], f32, kind="ExternalInput")
    biasB = nc.dram_tensor("biasB", [P, TPC], f32, kind="ExternalInput")
    c_idx = nc.dram_tensor("c_idx", [TPC, k_iters, P, J], i16, kind="ExternalInput")
    c_qsel = nc.dram_tensor("c_qsel", [TPC, k_iters, P, J], u8, kind="ExternalInput")
    p_idx = nc.dram_tensor("p_idx", [TPC, NPIXCH, P, J], i16, kind="ExternalInput")
    p_qsel = nc.dram_tensor("p_qsel", [TPC, NPIXCH, P, J], u8, kind="ExternalInput")
    pixout = nc.dram_tensor("pixout", [TPC, NPIXCH, P, J], f32, kind="ExternalOutput")

    with tile.TileContext(nc) as tc:
        with (
            tc.tile_pool(name="sb", bufs=1) as pool0,
            tc.tile_pool(name="dr", bufs=1, space="DRAM") as dpool,
        ):
            wg = pool0.tile([P, TPC, 17], f32, tag="wg")
            nc.sync.dma_start(wg[:], wgtB[:, :, :])
            bi = pool0.tile([P, TPC], f32, tag="bi")
            nc.sync.dma_start(bi[:], biasB[:, :])
            epsb = pool0.tile([P, 1], f32, tag="epsb")
            nc.vector.memset(epsb[:], 1e-10)
            hpib = pool0.tile([P, 1], f32, tag="hpib")
            nc.vector.memset(hpib[:], float(np.pi / 2))
            # iota over q: [128, 512, 8] fp32 with value q (0..7)
            iotaq = pool0.tile([P, J, 8], f32, tag="iotaq")
            nc.gpsimd.iota(iotaq[:], pattern=[[0, J], [1, 8]], base=0,
                           channel_multiplier=0,
                           allow_small_or_imprecise_dtypes=True)

            # ---------------- features -> s_j (= w of tree j), fp32
            s_tiles = []
            with tc.tile_pool(name="sbf", bufs=1) as poolf:
                for j in range(TPC):
                    at = poolf.tile([P, J, 15], f32, tag="at", bufs=2)
                    nc.sync.dma_start(at[:], attrs_sw[j])
                    df = poolf.tile([P, J], f32, tag="df", bufs=2)
                    nc.scalar.dma_start(df[:], diff_sw[j])
                    t9 = poolf.tile([P, J, 9], f32, tag="t9")
                    s9 = poolf.tile([P, J, 9], f32, tag="s9")
                    t1 = poolf.tile([P, J], f32, tag="t1")
                    t2 = poolf.tile([P, J], f32, tag="t2")
                    tm = poolf.tile([P, J], f32, tag="tm")
                    lg = poolf.tile([P, J], f32, tag="lg")

                    def wgb(kf, jj=j):
                        return wg[:, jj, kf:kf + 1].to_broadcast([P, J])

                    def acc(feat_ap, kf, first=False):
                        if first:
                            nc.vector.tensor_tensor(
                                out=lg[:], in0=feat_ap, in1=wgb(kf), op=Alu.mult)
                        else:
                            nc.vector.tensor_tensor(
                                out=tm[:], in0=feat_ap, in1=wgb(kf), op=Alu.mult)
                            nc.vector.tensor_tensor(
                                out=lg[:], in0=lg[:], in1=tm[:], op=Alu.add)

                    for k in range(4):
                        acc(at[:, :, k], k, first=(k == 0))
                    # area = log(a4)
                    nc.scalar.activation(t1[:], at[:, :, 4], Act.Ln)
                    acc(t1[:], 4)
                    # tail: log(|a|+eps)*sign(a), attrs 6..14
                    nc.scalar.activation(t9[:], at[:, :, 6:15], Act.Abs)
                    nc.scalar.activation(t9[:], t9[:], Act.Ln, bias=epsb[:, :])
                    nc.scalar.activation(s9[:], at[:, :, 6:15], Act.Sign)
                    nc.vector.tensor_tensor(
                        out=t9[:], in0=t9[:], in1=s9[:], op=Alu.mult)
                    for k in range(9):
                        acc(t9[:, :, k], 5 + k)
                    # lshape = sqrt(a7) / (sqrt(a6) + eps)
                    nc.scalar.activation(t1[:], at[:, :, 6], Act.Sqrt)
                    nc.scalar.activation(t1[:], t1[:], Act.Copy, bias=EPS)
                    nc.vector.reciprocal(t1[:], t1[:])
                    nc.scalar.activation(t2[:], at[:, :, 7], Act.Sqrt)
                    nc.vector.tensor_tensor(
                        out=t2[:], in0=t2[:], in1=t1[:], op=Alu.mult)
                    acc(t2[:], 14)
                    # cos, sin
                    nc.scalar.activation(t1[:], at[:, :, 5], Act.Sin,
                                         bias=hpib[:, :])
                    acc(t1[:], 15)
                    nc.scalar.activation(t2[:], at[:, :, 5], Act.Sin)
                    acc(t2[:], 16)
                    # + bias; sigmoid; w = diff * score
                    nc.vector.tensor_tensor(
                        out=lg[:], in0=lg[:],
                        in1=bi[:, j:j + 1].to_broadcast([P, J]), op=Alu.add)
                    nc.scalar.activation(lg[:], lg[:], Act.Sigmoid)
                    s_j = pool0.tile([P, J], f32, tag=f"s{j}")
                    nc.vector.tensor_tensor(
                        out=s_j[:], in0=lg[:], in1=df[:], op=Alu.mult)
                    s_tiles.append(s_j)

            # ---------------- chase + pixels
            _es = __import__("contextlib").ExitStack()
            pool = _es.enter_context(tc.tile_pool(name="sbc", bufs=1))

            tbl_bufs = []
            for tb in range(2):
                tbl_b = pool.tile([P, NE], f32, tag=f"tbl{tb}")
                tbl_bufs.append(tbl_b)

            def dump_and_load_table(j, s_ap, tagsfx):
                """state image [128,512] -> DRAM flat -> table [128,8192]."""
                tbl = tbl_bufs[j % 2]
                fl = dpool.tile([C], f32, tag=f"fl{j % 2}", bufs=2)
                nc.sync.dma_start(
                    fl[:].rearrange("(p x) -> p x", p=P), s_ap)
                flv = fl[:].rearrange("(E j) -> E j", E=8)
                for r in range(16):
                    eng = nc.sync if r % 2 == 0 else nc.scalar
                    eng.dma_start(tbl[8 * r:8 * r + 8, :], flv)
                return tbl

            def gather_round(tbl, idx_ap, qsel8_ap, out_ap, out_pred=None):
                """gather + transpose colocate + masked 8-way select.

                out_ap [128, 256, 2] view in slot layout; if out_pred given,
                select result is written there instead (for pixel rounds the
                final reduce writes contiguous [128,512])."""
                g = pool.tile([P, NE], f32, tag="g", bufs=1)
                nc.gpsimd.ap_gather(
                    out_ap=g[:], in_ap=tbl[:], idxs_ap=idx_ap,
                    channels=P, num_elems=NE, d=1, num_idxs=NE)
                gt = pool.tile([P, NE], f32, tag="gt", bufs=1)
                nc.vector.transpose(gt[:], g[:])
                qf = pool.tile([P, J], f32, tag="qf", bufs=1)
                nc.scalar.activation(qf[:], qsel8_ap, Act.Copy)
                mk = pool.tile([P, J, 8], f32, tag="mk", bufs=1)
                nc.vector.tensor_tensor(
                    out=mk[:], in0=qf[:].unsqueeze(2).to_broadcast([P, J, 8]),
                    in1=iotaq[:], op=Alu.is_equal)
                cand = gt[:].rearrange("p (c u q) -> p (c u) q", u=2, q=16)
                nc.vector.tensor_tensor(
                    out=mk[:], in0=mk[:], in1=cand[:, :, 0:8], op=Alu.mult)
                nc.vector.tensor_reduce(
                    out=out_ap, in_=mk[:],
                    axis=mybir.AxisListType.X, op=Alu.add)

            # chase: trees interleaved two-wide
            cidx_t = {}
            cq_t = {}
            for pair in (range(0, 2), range(2, 4)):
                for j in pair:
                    ci = pool.tile([P, k_iters, J], i16, tag=f"ci{j % 2}")
                    nc.scalar.dma_start(
                        ci[:], c_idx[j].rearrange("k p f -> p k f"))
                    cidx_t[j] = ci
                    cq = pool.tile([P, k_iters, J], u8, tag=f"cq{j % 2}")
                    nc.scalar.dma_start(
                        cq[:], c_qsel[j].rearrange("k p f -> p k f"))
                    cq_t[j] = cq
                for k in range(k_iters):
                    for j in pair:
                        s_j = s_tiles[j]
                        tbl = dump_and_load_table(j, s_j[:], f"c{j}k{k}")
                        sel = pool.tile([P, J], f32, tag="sel", bufs=1)
                        gather_round(
                            tbl, cidx_t[j][:, k, :], cq_t[j][:, k, :], sel[:])
                        nc.vector.tensor_tensor(
                            out=s_j[:], in0=s_j[:], in1=sel[:], op=Alu.add)

            # pixels: val = s/SCALING; one table per tree, 4 chunks
            pidx_t = {}
            pq_t = {}
            for pair in (range(0, 2), range(2, 4)):
                for j in pair:
                    nc.vector.tensor_scalar_mul(
                        s_tiles[j][:], s_tiles[j][:], 1.0 / cfg["SCALING"])
                    pi = pool.tile([P, NPIXCH, J], i16, tag=f"pi{j % 2}")
                    nc.scalar.dma_start(
                        pi[:], p_idx[j].rearrange("k p f -> p k f"))
                    pidx_t[j] = pi
                    pq = pool.tile([P, NPIXCH, J], u8, tag=f"pq{j % 2}")
                    nc.scalar.dma_start(
                        pq[:], p_qsel[j].rearrange("k p f -> p k f"))
                    pq_t[j] = pq
                for j in pair:
                    tbl = dump_and_load_table(j, s_tiles[j][:], f"p{j}")
                    for c in range(NPIXCH):
                        po = pool.tile([P, J], f32, tag="po", bufs=2)
                        gather_round(
                            tbl, pidx_t[j][:, c, :], pq_t[j][:, c, :], po[:])
                        nc.sync.dma_start(pixout[j, c], po[:])
            _es.close()

    nc.compile()
    return nc


_CACHE = {}
TRACE = False
LAST_RESULT = None


def _get_nc(cfg, k_iters):
    key = (k_iters,)
    if key not in _CACHE:
        _CACHE[key] = _build(cfg, k_iters)
    return _CACHE[key]


def kernel(diff, attrs, weight, bias, parent, pix2cc):
    cfg = CFG
    diff = np.ascontiguousarray(np.asarray(diff, np.float32))
    attrs = np.ascontiguousarray(np.asarray(attrs, np.float32))
    weight = np.ascontiguousarray(np.asarray(weight, np.float32))
    bias = np.ascontiguousarray(np.asarray(bias, np.float32))
    parent = np.ascontiguousarray(np.asarray(parent, np.int32))
    pix2cc = np.ascontiguousarray(np.asarray(pix2cc, np.int32))

    in_maps, k_iters = _host_prep(
        cfg, diff, attrs, weight, bias, parent, pix2cc)
    nc = _get_nc(cfg, k_iters)
    res = run_bass_kernel_spmd(
        nc, in_maps, core_ids=list(range(cfg["NCORES"])), trace=TRACE)
    global LAST_RESULT
    LAST_RESULT = res
    return _host_assemble(cfg, res.results)
